# revision 1
# baseline (speedup 1.0000x reference)
"""Multi-head attention (B=4, S=2048, E=768, H=12, Dh=64) on 8 TRN2 NeuronCores.

Sharding: batch x head-group tensor parallel. Core c handles batch b = c//2 and
head group g = c%2 (6 heads each). Each core computes its heads' Q/K/V
projections, full attention over the 2048-token sequence, and a partial
out-projection over its 384 concat-features. The host sums the two partials per
batch and adds the output bias.

Device layout notes:
 - Host pre-transposes activations to x^T [E, S] and casts to bf16, so the
   contraction dim (E) lands on SBUF partitions with contiguous DMA loads.
 - Q^T/K^T are produced feature-major [384, S] (head pairs per 128-partition
   tile); V is token-major, each head augmented with 64 ones columns so the
   PV matmul emits the softmax denominator replicated on psum partitions
   64-127 (normalization is then one DVE reciprocal + one multiply-cast).
 - Scores are computed transposed (S^T tiles [128 keys, S queries]) and
   exponentiated on ScalarE straight out of PSUM (no max-subtraction: logits
   are ~N(0, 0.3), so exp is numerically safe, matching softmax exactly).
 - PSUM (8 banks) holds proj (2) + double-buffered S^T (4) + ctx (2)
   concurrently; the PE stream is software-pipelined by hand (next score
   tile issued before the current context matmul) and projection /
   out-projection work is injected into the exp-bound attention windows.
"""

import math
import os
import sys
from contextlib import ExitStack

import numpy as np

for _p in ("/opt/trn_rl_repo", "/root/.axon_site/_ro/trn_rl_repo"):
    if os.path.isdir(_p) and _p not in sys.path:
        sys.path.append(_p)

# NTFF tracing hooks (antenv.axon_hooks) don't exist in this container;
# make sure an ambient BASS_TRACE can't route execution into that path.
os.environ["BASS_NEVER_TRACE"] = "1"

import ml_dtypes  # noqa: E402

import concourse.bass as bass  # noqa: E402
import concourse.tile as tile  # noqa: E402
from concourse import bacc, mybir  # noqa: E402
from concourse.bass_utils import run_bass_kernel_spmd  # noqa: E402

BF16 = mybir.dt.bfloat16
F32 = mybir.dt.float32
NP_BF16 = ml_dtypes.bfloat16

B, S, E, H, DH = 4, 2048, 768, 12, 64
N_CORES = 8
G = H // 2  # heads per core (6)


def build_nc(T=S, EMB=E, NH=G, dh=DH, OUT=E, trace_label=""):
    """Emit the per-core Bass/Tile program. All cores run this same program.

    T: sequence length; EMB: model dim; NH: heads on this core (even);
    dh: head dim (64); OUT: out-projection output width.
    """
    assert T % 128 == 0 and EMB % 128 == 0 and dh == 64 and NH % 2 == 0
    FEAT = NH * dh
    assert FEAT % 128 == 0
    EC = EMB // 128  # contraction chunks for projections
    TT = T // 128  # token tiles
    FT = FEAT // 128  # feature tiles (head pairs)
    SCH = min(512, T)  # matmul moving free-dim chunk
    NSCH = T // SCH
    T2 = max(128, T // 2)  # attention query-half width (2 PSUM banks)
    NSH = T // T2  # query halves per head
    SCH2 = min(512, T2)
    NSCH2 = T2 // SCH2
    _ock = OUT // 2 if 128 < OUT <= 1024 and OUT % 2 == 0 else 512
    OCHUNKS = [(o, min(_ock, OUT - o)) for o in range(0, OUT, _ock)]
    scale = 1.0 / math.sqrt(dh)

    nc = bacc.Bacc("TRN2", target_bir_lowering=False, debug=False, num_devices=N_CORES)

    # ---- DRAM I/O ----
    xqT_d = nc.dram_tensor("xqT", [EMB, T], BF16, kind="ExternalInput").ap()
    xkT_d = nc.dram_tensor("xkT", [EMB, T], BF16, kind="ExternalInput").ap()
    xvT_d = nc.dram_tensor("xvT", [EMB, T], BF16, kind="ExternalInput").ap()
    wq_d = nc.dram_tensor("wq", [EMB, FEAT], BF16, kind="ExternalInput").ap()
    wk_d = nc.dram_tensor("wk", [EMB, FEAT], BF16, kind="ExternalInput").ap()
    wv_d = nc.dram_tensor("wv", [EMB, FEAT], BF16, kind="ExternalInput").ap()
    wo_d = nc.dram_tensor("wo", [FEAT, OUT], BF16, kind="ExternalInput").ap()
    bq_d = nc.dram_tensor("bq", [1, FEAT], BF16, kind="ExternalInput").ap()
    bk_d = nc.dram_tensor("bk", [1, FEAT], BF16, kind="ExternalInput").ap()
    bv_d = nc.dram_tensor("bv", [1, FEAT], BF16, kind="ExternalInput").ap()
    out_d = nc.dram_tensor("out", [T, OUT], F32, kind="ExternalOutput").ap()

    with tile.TileContext(nc) as tc, ExitStack() as ctx:
        persist = ctx.enter_context(tc.tile_pool(name="persist", bufs=1))

        # ---- persistent SBUF tensors ----
        wq_sb = [persist.tile([128, FEAT], BF16, tag=f"wq{j}", name=f"wq{j}") for j in range(EC)]
        wk_sb = [persist.tile([128, FEAT], BF16, tag=f"wk{j}", name=f"wk{j}") for j in range(EC)]
        wv_sb = [persist.tile([128, FEAT], BF16, tag=f"wv{j}", name=f"wv{j}") for j in range(EC)]
        wo_sb = [persist.tile([128, OUT], BF16, tag=f"wo{j}", name=f"wo{j}") for j in range(FT)]
        bq_sb = persist.tile([1, FEAT], BF16, tag="bq", name="bq")
        bk_sb = persist.tile([1, FEAT], BF16, tag="bk", name="bk")
        bv_sb = persist.tile([1, FEAT], BF16, tag="bv", name="bv")
        ones_row = persist.tile([1, T], BF16, tag="ones_row", name="ones_row")
        xqT_sb = [persist.tile([128, T], BF16, tag=f"xq{j}", name=f"xq{j}") for j in range(EC)]
        xkT_sb = [persist.tile([128, T], BF16, tag=f"xk{j}", name=f"xk{j}") for j in range(EC)]
        xvT_sb = [persist.tile([128, T], BF16, tag=f"xv{j}", name=f"xv{j}") for j in range(EC)]
        qT_sb = [persist.tile([128, T], BF16, tag=f"qT{j}", name=f"qT{j}") for j in range(FT)]
        kT_sb = [persist.tile([128, T], BF16, tag=f"kT{j}", name=f"kT{j}") for j in range(FT)]
        # V token-major, each head augmented with 64 ones columns so the PV
        # matmul emits the softmax denominator replicated on partitions 64-127
        v_sb = [persist.tile([128, NH * (dh + 64)], BF16, tag=f"v{i}", name=f"v{i}") for i in range(TT)]
        cn_sb = [persist.tile([128, T], BF16, tag=f"cn{j}", name=f"cn{j}") for j in range(FT)]

        # ---- weight/bias/x loads (Q/K path first: it gates head 0) ----
        nc.sync.dma_start(bq_sb[:], bq_d[:])
        nc.sync.dma_start(bk_sb[:], bk_d[:])
        for j in range(EC):
            nc.sync.dma_start(wq_sb[j][:], wq_d[j * 128 : (j + 1) * 128, :])
            nc.sync.dma_start(xqT_sb[j][:], xqT_d[j * 128 : (j + 1) * 128, :])
            nc.sync.dma_start(wk_sb[j][:], wk_d[j * 128 : (j + 1) * 128, :])
            nc.sync.dma_start(xkT_sb[j][:], xkT_d[j * 128 : (j + 1) * 128, :])
        nc.sync.dma_start(bv_sb[:], bv_d[:])
        for j in range(EC):
            nc.sync.dma_start(wv_sb[j][:], wv_d[j * 128 : (j + 1) * 128, :])
            nc.sync.dma_start(xvT_sb[j][:], xvT_d[j * 128 : (j + 1) * 128, :])
        for j in range(FT):
            nc.sync.dma_start(wo_sb[j][:], wo_d[j * 128 : (j + 1) * 128, :])
        nc.vector.memset(ones_row[:], 1.0)
        # ones columns of augmented V (written once)
        for i in range(TT):
            vview = v_sb[i][:].rearrange("p (h x) -> p h x", x=dh + 64)
            nc.vector.memset(vview[:, :, dh:], 1.0)

        # ---- compute: projections + attention + out-projection ----
        # PSUM budget (8 banks): proj 2 (bufs=2 x 1 bank) + ST 4 (bufs=2 x 2)
        # + ctx 2 (bufs=1 x 2). Everything coexists, so Tile can overlap the
        # phases; PE instruction order is software-pipelined by hand.
        with (
            tc.tile_pool(name="ppsum", bufs=2, space="PSUM") as ppool,
            tc.tile_pool(name="stpsum", bufs=2, space="PSUM") as stpool,
            tc.tile_pool(name="ctpsum", bufs=1, space="PSUM") as ctpool,
            tc.tile_pool(name="ptpool", bufs=5) as ptpool,
            tc.tile_pool(name="normpool", bufs=3) as npool,
            tc.tile_pool(name="outsb", bufs=4) as osbpool,
        ):

            def proj_qk(j, part=None, nparts=1):
                groups = [
                    (t, n)
                    for t in range(2)
                    for n in range(NSCH)
                ]
                if part is not None:
                    groups = groups[
                        (len(groups) * part) // nparts : (len(groups) * (part + 1)) // nparts
                    ]
                qk = (
                    (wq_sb, bq_sb, xqT_sb, qT_sb),
                    (wk_sb, bk_sb, xkT_sb, kT_sb),
                )
                for t, n in groups:
                    w_sb, b_sb, x_sb, dst = qk[t]
                    if True:
                        ps = ppool.tile([128, SCH], F32, tag="proj", name="proj")
                        # bias first (K=1 rank-1 update): depends only on the
                        # tiny bias DMA, so the group starts before x arrives
                        nc.tensor.matmul(
                            ps[:],
                            b_sb[:, j * 128 : (j + 1) * 128],
                            ones_row[:, 0:SCH],
                            start=True,
                            stop=False,
                        )
                        for e in range(EC):
                            nc.tensor.matmul(
                                ps[:],
                                w_sb[e][:, j * 128 : (j + 1) * 128],
                                x_sb[e][:, n * SCH : (n + 1) * SCH],
                                start=False,
                                stop=(e == EC - 1),
                            )
                        nc.vector.tensor_copy(dst[j][:, n * SCH : (n + 1) * SCH], ps[:])

            def proj_v(tiles=None):
                for i in tiles if tiles is not None else range(TT):
                    ps = ppool.tile([128, FEAT], F32, tag="proj", name="proj")
                    nc.tensor.matmul(
                        ps[:], ones_row[:, 0:128], bv_sb[:], start=True, stop=False
                    )
                    for e in range(EC):
                        nc.tensor.matmul(
                            ps[:],
                            xvT_sb[e][:, i * 128 : (i + 1) * 128],
                            wv_sb[e][:],
                            start=False,
                            stop=(e == EC - 1),
                        )
                    dst = v_sb[i][:].rearrange("p (h x) -> p h x", x=dh + 64)[:, :, 0:dh]
                    srcv = ps[:].rearrange("p (h d) -> p h d", d=dh)
                    nc.vector.tensor_copy(dst, srcv)

            def st_tile(i, kT_h, qT_h, s0):
                st = stpool.tile([128, T2], F32, tag="st", name="st")
                for n in range(NSCH2):
                    nc.tensor.matmul(
                        st[:, n * SCH2 : (n + 1) * SCH2],
                        kT_h[:, i * 128 : (i + 1) * 128],
                        qT_h[:, s0 + n * SCH2 : s0 + (n + 1) * SCH2],
                        start=True,
                        stop=True,
                    )
                return st

            pending_sts = []

            def head_args(h, sh):
                ft, half = h // 2, (h % 2) * 64
                return (
                    kT_sb[ft][half : half + 64, :],
                    qT_sb[ft][half : half + 64, :],
                    sh * T2,
                )

            def head(h, sh, filler=None, nxt=None):
                # keeps 2 score tiles in flight and pre-issues the NEXT
                # head's first 2 before this head's last context matmul, so
                # the ScalarE exp stream never stalls at head boundaries
                ft, half = h // 2, (h % 2) * 64
                kT_h, qT_h, s0 = head_args(h, sh)
                ct = ctpool.tile([128, T2], F32, tag="ct", name="ct")
                sts = pending_sts[:]
                del pending_sts[:]
                while len(sts) < min(2, TT):
                    sts.append(st_tile(len(sts), kT_h, qT_h, s0))
                nissued = 0
                for i in range(TT):
                    st = sts.pop(0)
                    pt = ptpool.tile([128, T2], BF16, tag="pt", name="pt")
                    nc.scalar.activation(
                        pt[:], st[:], mybir.ActivationFunctionType.Exp, scale=scale
                    )
                    if i + 2 < TT:
                        sts.append(st_tile(i + 2, kT_h, qT_h, s0))
                    elif nxt is not None and nissued < min(2, TT):
                        pending_sts.append(st_tile(nissued, *head_args(*nxt)))
                        nissued += 1
                    if filler is not None:
                        filler(i)
                    for n in range(NSCH2):
                        nc.tensor.matmul(
                            ct[:, n * SCH2 : (n + 1) * SCH2],
                            v_sb[i][:, h * (dh + 64) : (h + 1) * (dh + 64)],
                            pt[:, n * SCH2 : (n + 1) * SCH2],
                            start=(i == 0),
                            stop=(i == TT - 1),
                        )

                # normalize: cn[f, s] = ct[f, s] * (1 / ct[64.., s])
                recip = npool.tile([64, T2], F32, tag="recip", name="recip")
                nc.vector.reciprocal(recip[:], ct[64:128, :])
                nc.vector.tensor_tensor(
                    cn_sb[ft][half : half + 64, s0 : s0 + T2],
                    ct[0:64, :],
                    recip[:],
                    op=mybir.AluOpType.mult,
                )

            def outproj(tiles):
                for i in tiles:
                    osb = osbpool.tile([128, OUT], F32, tag="osb", name="osb")
                    for oc, ow in OCHUNKS:
                        ps = ppool.tile([128, ow], F32, tag="proj", name="proj")
                        for f in range(FT):
                            nc.tensor.matmul(
                                ps[:],
                                cn_sb[f][:, i * 128 : (i + 1) * 128],
                                wo_sb[f][:, oc : oc + ow],
                                start=(f == 0),
                                stop=(f == FT - 1),
                            )
                        nc.vector.tensor_copy(osb[:, oc : oc + ow], ps[:])
                    nc.sync.dma_start(out_d[i * 128 : (i + 1) * 128, :], osb[:])

            proj_qk(0)
            # pre-issue head 0's first score tiles BEFORE any V work: V
            # depends on the last-arriving xvT DMAs and must not gate exp_0
            for z in range(min(2, TT)):
                pending_sts.append(st_tile(z, *head_args(0, 0)))
            # V tile i is first needed at head 0's CT step i: emit tile 0/1
            # up front and drip the rest into head 0's pipeline
            proj_v(range(2))

            def v_filler(i):
                if i + 2 < TT:
                    proj_v([i + 2])

            half_tiles = T2 // 128 if NSH == 2 else 0
            seq = [
                (2 * p + z, sh)
                for p in range(NH // 2)
                for sh in range(NSH)
                for z in (0, 1)
            ]
            pos = 0
            for p in range(NH // 2):
                last = 2 * p + 1 == NH - 1
                for sh in range(NSH):
                    nxt = seq[pos + 1] if pos + 1 < len(seq) else None
                    head(2 * p, sh, v_filler if (p, sh) == (0, 0) else None, nxt=nxt)
                    pos += 1
                    # spread the next feature-tile's projections over this
                    # pair's ACT-bound windows (3 injection points)
                    if p + 1 < FT and NSH == 2:
                        proj_qk(p + 1, part=2 * sh, nparts=3)
                    if last and sh == 1 and NSH == 2:
                        outproj(range(half_tiles // 2, half_tiles))
                    nxt = seq[pos + 1] if pos + 1 < len(seq) else None
                    head(2 * p + 1, sh, nxt=nxt)
                    pos += 1
                    if p + 1 < FT and NSH == 2 and sh == 0:
                        proj_qk(p + 1, part=1, nparts=3)
                    if p + 1 < FT and NSH == 1:
                        proj_qk(p + 1)
                    if last and sh == 0 and NSH == 2:
                        # heads done for queries [0, T2): drip their out-proj
                        # tiles into the remaining windows
                        outproj(range(half_tiles // 2))
            outproj(range(half_tiles, TT))

    nc.compile()
    return nc


def shard_inputs(query, key, value, wq, bq, wk, bk, wv, bv, wo):
    """Build the 8 per-core input maps (host-side cast/transpose/slice)."""
    in_maps = []
    xT = {}
    for b in range(B):
        xT[b] = (
            np.ascontiguousarray(query[b].T).astype(NP_BF16),
            np.ascontiguousarray(key[b].T).astype(NP_BF16),
            np.ascontiguousarray(value[b].T).astype(NP_BF16),
        )
    gw = {}
    for g in range(2):
        hs = slice(g * G, (g + 1) * G)
        gw[g] = dict(
            wq=np.ascontiguousarray(wq[hs].transpose(1, 0, 2).reshape(E, G * DH)).astype(NP_BF16),
            wk=np.ascontiguousarray(wk[hs].transpose(1, 0, 2).reshape(E, G * DH)).astype(NP_BF16),
            wv=np.ascontiguousarray(wv[hs].transpose(1, 0, 2).reshape(E, G * DH)).astype(NP_BF16),
            wo=np.ascontiguousarray(wo[g * G * DH : (g + 1) * G * DH, :]).astype(NP_BF16),
            bq=np.ascontiguousarray(bq[hs].reshape(1, G * DH)).astype(NP_BF16),
            bk=np.ascontiguousarray(bk[hs].reshape(1, G * DH)).astype(NP_BF16),
            bv=np.ascontiguousarray(bv[hs].reshape(1, G * DH)).astype(NP_BF16),
        )
    for c in range(N_CORES):
        b, g = c // 2, c % 2
        m = dict(xqT=xT[b][0], xkT=xT[b][1], xvT=xT[b][2])
        m.update(gw[g])
        in_maps.append(m)
    return in_maps


_CACHED_NC = None


def kernel(query, key, value, wq, bq, wk, bk, wv, bv, wo, bo):
    global _CACHED_NC
    query, key, value = (np.asarray(a, np.float32) for a in (query, key, value))
    wq, bq, wk, bk, wv, bv, wo, bo = (
        np.asarray(a, np.float32) for a in (wq, bq, wk, bk, wv, bv, wo, bo)
    )
    in_maps = shard_inputs(query, key, value, wq, bq, wk, bk, wv, bv, wo)
    if _CACHED_NC is None:
        _CACHED_NC = build_nc()
    res = run_bass_kernel_spmd(_CACHED_NC, in_maps, list(range(N_CORES)))
    out = np.empty((B, S, E), np.float32)
    for b in range(B):
        out[b] = res.results[2 * b]["out"] + res.results[2 * b + 1]["out"] + bo[None, :]
    return out



# revision 10
# speedup vs baseline: 1.0610x; 1.0610x over previous
"""Multi-head attention (B=4, S=2048, E=768, H=12, Dh=64) on 8 TRN2 NeuronCores.

Sharding: batch x head-group tensor parallel. Core c handles batch b = c//2 and
head group g = c%2 (6 heads each). Each core computes its heads' Q/K/V
projections, full attention over the 2048-token sequence, and a partial
out-projection over its 384 concat-features. The host sums the two partials per
batch and adds the output bias.

fp8 DoubleRow design (all matmul inputs fp8e4 except the out-projection):
 - Projections contract E=768 as 3 plane-pairs per term with residual
   compensation: q = x8.w8 + x8.wr8 + xr8.w8 (w pre-scaled x64 on host so fp8
   sees a normal range; the 1/64 and the bias fold into the PSUM->SBUF copy).
 - Q/K land directly in fp8 "dh-split" layout: head h occupies 32 partitions,
   planes = dh halves, so the dh=64 score contraction is ONE DoubleRow matmul
   per 128-key x 512-query block (heads 0-3 in a quad tile at partition 32h,
   heads 4-5 in a duo tile).
 - exp runs on ACT emitting fp8 straight into paired P tiles (planes = key
   chunks); designated chunks are offloaded to DVE/Pool via the Schraudolph
   int32 bitcast exp to keep ACT off the critical path.
 - PV pairs key chunks per DoubleRow matmul; V is fp8 with an additive fp8
   residual pass (vr8 = v - fp8(v)), recovering ~bf16 accuracy at half the
   bf16 PE cost. V is ones-augmented so the matmul emits softmax denominators
   on PSUM partitions 64-127.
 - Out-projection stays bf16; partial outputs ship as bf16 and are upcast and
   summed on the host.
"""

import math
import os
import sys
from contextlib import ExitStack

import numpy as np

for _p in ("/opt/trn_rl_repo", "/root/.axon_site/_ro/trn_rl_repo"):
    if os.path.isdir(_p) and _p not in sys.path:
        sys.path.append(_p)

os.environ["BASS_NEVER_TRACE"] = "1"

import ml_dtypes  # noqa: E402

import concourse.bass as bass  # noqa: E402
import concourse.tile as tile  # noqa: E402
from concourse import bacc, mybir  # noqa: E402
from concourse.bass_utils import run_bass_kernel_spmd  # noqa: E402

BF16 = mybir.dt.bfloat16
F32 = mybir.dt.float32
F8 = mybir.dt.float8e4
I32 = mybir.dt.int32
NP_BF16 = ml_dtypes.bfloat16
NP_F8 = ml_dtypes.float8_e4m3

B, S, E, H, DH = 4, 2048, 768, 12, 64
N_CORES = 8
G = H // 2  # heads per core (6)
WSCALE = 64.0  # host pre-scale on projection weights (fp8 range)

# exp chunk offload: per (head, half), these key-chunk indices run via
# Schraudolph instead of ACT exp. GPSIMD cannot touch PSUM, so "POOL"
# chunks split the work: DVE does the PSUM->int32 scale step, Pool does
# the SBUF bitcast->fp8 copy. "DVE" chunks run both steps on DVE.
SCHR_DVE = ()
SCHR_POOL = (4, 5, 10, 11)
SCHR_A = 2**23 / math.log(2)
SCHR_C = float(127 * 2**23 - 0.043677 * 2**23)


def build_nc(T=S, EMB=E, NH=G, dh=DH, OUT=E, trace_label=""):
    """Emit the per-core Bass/Tile program. All cores run this same program."""
    assert T % 128 == 0 and EMB % 256 == 0 and dh == 64 and NH == 6
    FEAT = NH * dh  # 384
    EP = EMB // 256  # E plane-pairs (3)
    TT = T // 128  # key/token tiles (16)
    KP = TT // 2  # key-chunk pairs (8)
    SCH = 512  # projection N chunk
    NSCH = T // SCH
    T2 = T // 2  # attention query-half width (2 PSUM banks)
    NSH = 2
    SCH2 = 512
    NSCH2 = T2 // SCH2
    OCHUNKS = [(0, OUT // 2), (OUT // 2, OUT // 2)]
    scale = 1.0 / math.sqrt(dh)
    VW = dh + 64  # per-head augmented V width

    # Q/K projection groups: (name, feature col offset, width, plane, qk)
    # quad: heads 0-3, duo: heads 4-5; plane = dh half
    QK_GROUPS = [
        ("qA", 0, 128, 0, 0), ("kA", 0, 128, 0, 1),
        ("qB", 128, 128, 1, 0), ("kB", 128, 128, 1, 1),
        ("qC", 256, 64, 0, 0), ("kC", 256, 64, 0, 1),
        ("qD", 320, 64, 1, 0), ("kD", 320, 64, 1, 1),
    ]

    nc = bacc.Bacc("TRN2", target_bir_lowering=False, debug=False, num_devices=N_CORES)

    # ---- DRAM I/O ----
    x8_d, xr8_d, w8_d, wr8_d = {}, {}, {}, {}
    for t in ("q", "k", "v"):
        x8_d[t] = nc.dram_tensor(f"x{t}8", [EMB // 2, 2 * T], F8, kind="ExternalInput").ap()
        xr8_d[t] = nc.dram_tensor(f"x{t}r8", [EMB // 2, 2 * T], F8, kind="ExternalInput").ap()
        w8_d[t] = nc.dram_tensor(f"w{t}8", [EMB // 2, 2 * FEAT], F8, kind="ExternalInput").ap()
        wr8_d[t] = nc.dram_tensor(f"w{t}r8", [EMB // 2, 2 * FEAT], F8, kind="ExternalInput").ap()
    wo_d = nc.dram_tensor("wo", [FEAT, OUT], BF16, kind="ExternalInput").ap()
    bqk_d = nc.dram_tensor("bqk", [128, 8], F32, kind="ExternalInput").ap()
    bv_d = nc.dram_tensor("bv", [1, FEAT], BF16, kind="ExternalInput").ap()
    out_d = nc.dram_tensor("out", [T, OUT], BF16, kind="ExternalOutput").ap()

    with tile.TileContext(nc) as tc, ExitStack() as ctx:
        persist = ctx.enter_context(tc.tile_pool(name="persist", bufs=1))

        # ---- persistent SBUF tensors ----
        x8_sb, xr8_sb, w8_sb, wr8_sb = {}, {}, {}, {}
        for t in ("q", "k", "v"):
            x8_sb[t] = [persist.tile([128, 2 * T], F8, tag=f"x{t}8_{j}", name=f"x{t}8_{j}") for j in range(EP)]
            xr8_sb[t] = [persist.tile([128, 2 * T], F8, tag=f"x{t}r8_{j}", name=f"x{t}r8_{j}") for j in range(EP)]
            w8_sb[t] = [persist.tile([128, 2 * FEAT], F8, tag=f"w{t}8_{j}", name=f"w{t}8_{j}") for j in range(EP)]
            wr8_sb[t] = [persist.tile([128, 2 * FEAT], F8, tag=f"w{t}r8_{j}", name=f"w{t}r8_{j}") for j in range(EP)]
        wo_sb = [persist.tile([128, OUT], BF16, tag=f"wo{j}", name=f"wo{j}") for j in range(FEAT // 128)]
        bqk_sb = persist.tile([128, 8], F32, tag="bqk", name="bqk")
        bv_sb = persist.tile([1, FEAT], BF16, tag="bv", name="bv")
        ones_row = persist.tile([1, 128], BF16, tag="ones_row", name="ones_row")
        # Q/K fp8 dh-split tiles: quad = heads 0-3 (partition 32h+d),
        # duo = heads 4-5; planes (free-major halves) = dh halves
        q8_quad = persist.tile([128, 2 * T], F8, tag="q8_quad", name="q8_quad")
        q8_duo = persist.tile([64, 2 * T], F8, tag="q8_duo", name="q8_duo")
        k8_quad = persist.tile([128, 2 * T], F8, tag="k8_quad", name="k8_quad")
        k8_duo = persist.tile([64, 2 * T], F8, tag="k8_duo", name="k8_duo")
        # V fp8 pair tiles (planes = key chunks 2j / 2j+1), per-head layout
        # [NH, 64 v | 64 ones]; vr8 = residual (ones slots zero)
        v8_sb = [persist.tile([128, 2 * NH * VW], F8, tag=f"v8_{j}", name=f"v8_{j}") for j in range(KP)]
        vr8_sb = [persist.tile([128, 2 * NH * VW], F8, tag=f"vr8_{j}", name=f"vr8_{j}") for j in range(KP)]
        cn_sb = [persist.tile([128, T], BF16, tag=f"cn{j}", name=f"cn{j}") for j in range(FEAT // 128)]

        def pair(ap):  # [p, (2 n)] -> [p, 2, n]
            return ap.rearrange("p (two n) -> p two n", two=2)

        # ---- DMA loads (Q/K path first: it gates head 0) ----
        nc.sync.dma_start(bqk_sb[:], bqk_d[:])
        for t in ("q", "k"):
            for j in range(EP):
                nc.sync.dma_start(w8_sb[t][j][:], w8_d[t][j * 128 : (j + 1) * 128, :])
                nc.sync.dma_start(x8_sb[t][j][:], x8_d[t][j * 128 : (j + 1) * 128, :])
                nc.sync.dma_start(wr8_sb[t][j][:], wr8_d[t][j * 128 : (j + 1) * 128, :])
                nc.sync.dma_start(xr8_sb[t][j][:], xr8_d[t][j * 128 : (j + 1) * 128, :])
        nc.sync.dma_start(bv_sb[:], bv_d[:])
        for j in range(EP):
            nc.sync.dma_start(w8_sb["v"][j][:], w8_d["v"][j * 128 : (j + 1) * 128, :])
            nc.sync.dma_start(x8_sb["v"][j][:], x8_d["v"][j * 128 : (j + 1) * 128, :])
            nc.sync.dma_start(wr8_sb["v"][j][:], wr8_d["v"][j * 128 : (j + 1) * 128, :])
            nc.sync.dma_start(xr8_sb["v"][j][:], xr8_d["v"][j * 128 : (j + 1) * 128, :])
        for j in range(FEAT // 128):
            nc.sync.dma_start(wo_sb[j][:], wo_d[j * 128 : (j + 1) * 128, :])
        nc.vector.memset(ones_row[:], 1.0)
        for j in range(KP):
            v8v = v8_sb[j][:].rearrange("p (two h x) -> p two h x", two=2, x=VW)
            nc.vector.memset(v8v[:, :, :, dh:], 1.0)
            vr8v = vr8_sb[j][:].rearrange("p (two h x) -> p two h x", two=2, x=VW)
            nc.vector.memset(vr8v[:, :, :, dh:], 0.0)

        with (
            tc.tile_pool(name="ppsum", bufs=2, space="PSUM") as ppool,
            tc.tile_pool(name="stpsum", bufs=2, space="PSUM") as stpool,
            tc.tile_pool(name="ctpsum", bufs=1, space="PSUM") as ctpool,
            tc.tile_pool(name="ptpool", bufs=4) as ptpool,
            tc.tile_pool(name="i32pool", bufs=2) as ipool,
            tc.tile_pool(name="normpool", bufs=3) as npool,
            tc.tile_pool(name="outsb", bufs=4) as osbpool,
        ):
            qk_dst = {
                0: (q8_quad, q8_duo),
                1: (k8_quad, k8_duo),
            }

            def proj_qk(gi, chunks=None):
                """One Q/K projection group (all its N chunks by default)."""
                name, coff, width, plane, qk = QK_GROUPS[gi]
                t = "q" if qk == 0 else "k"
                for n in chunks if chunks is not None else range(NSCH):
                    ps = ppool.tile([width, SCH], F32, tag="proj", name=name)
                    terms = [
                        (x8_sb[t], w8_sb[t]),
                        (x8_sb[t], wr8_sb[t]),
                        (xr8_sb[t], w8_sb[t]),
                    ]
                    for ti, (xs, ws) in enumerate(terms):
                        for j in range(EP):
                            nc.tensor.matmul(
                                ps[:],
                                pair(ws[j][:])[:, :, coff : coff + width],
                                pair(xs[j][:])[:, :, n * SCH : (n + 1) * SCH],
                                start=(ti == 0 and j == 0),
                                stop=(ti == 2 and j == EP - 1),
                                perf_mode=mybir.MatmulPerfMode.DoubleRow,
                            )
                    # copy with bias fold + 1/WSCALE: out = (ps + b*W)/W
                    quad, duo = qk_dst[qk]
                    dst = quad if width == 128 else duo
                    dv = pair(dst[:])[
                        :width, plane, n * SCH : (n + 1) * SCH
                    ]
                    nc.vector.tensor_scalar(
                        dv,
                        ps[:],
                        bqk_sb[:width, gi : gi + 1],
                        1.0 / WSCALE,
                        op0=mybir.AluOpType.add,
                        op1=mybir.AluOpType.mult,
                    )

            def proj_v(tiles):
                """V projection for token tiles; writes v8 + vr8 pair tiles."""
                for i in tiles:
                    ps = ppool.tile([128, FEAT], F32, tag="proj", name="vproj")
                    # bias (pre-scaled x64 on host): rank-1 ones x bv
                    nc.tensor.matmul(
                        ps[:], ones_row[:, 0:128], bv_sb[:], start=True, stop=False
                    )
                    terms = [
                        (x8_sb["v"], w8_sb["v"]),
                        (x8_sb["v"], wr8_sb["v"]),
                        (xr8_sb["v"], w8_sb["v"]),
                    ]
                    for ti, (xs, ws) in enumerate(terms):
                        for j in range(EP):
                            nc.tensor.matmul(
                                ps[:],
                                pair(xs[j][:])[:, :, i * 128 : (i + 1) * 128],
                                pair(ws[j][:]),
                                start=False,
                                stop=(ti == 2 and j == EP - 1),
                                perf_mode=mybir.MatmulPerfMode.DoubleRow,
                            )
                    j, pl = i // 2, i % 2
                    psv = ps[:].rearrange("p (h d) -> p h d", d=dh)
                    v8v = v8_sb[j][:].rearrange(
                        "p (two h x) -> p two h x", two=2, x=VW
                    )[:, pl, :, 0:dh]
                    vr8v = vr8_sb[j][:].rearrange(
                        "p (two h x) -> p two h x", two=2, x=VW
                    )[:, pl, :, 0:dh]
                    # v8 = fp8(ps/W); vr8 = fp8(ps/W - v8)
                    nc.vector.tensor_scalar(
                        v8v, psv, 1.0 / WSCALE, None, op0=mybir.AluOpType.mult
                    )
                    nc.vector.scalar_tensor_tensor(
                        vr8v,
                        psv,
                        1.0 / WSCALE,
                        v8v,
                        op0=mybir.AluOpType.mult,
                        op1=mybir.AluOpType.subtract,
                    )

            def st_tile(i, h, sh):
                """Transposed score tile: keys [128i..) x queries half sh."""
                if h < 4:
                    kq = (k8_quad, q8_quad)
                    base = 32 * h
                else:
                    kq = (k8_duo, q8_duo)
                    base = 32 * (h - 4)
                kt, qt = kq
                s0 = sh * T2
                st = stpool.tile([128, T2], F32, tag="st", name="st")
                for n in range(NSCH2):
                    nc.tensor.matmul(
                        st[:, n * SCH2 : (n + 1) * SCH2],
                        pair(kt[:])[base : base + 32, :, i * 128 : (i + 1) * 128],
                        pair(qt[:])[
                            base : base + 32, :, s0 + n * SCH2 : s0 + (n + 1) * SCH2
                        ],
                        start=True,
                        stop=True,
                        perf_mode=mybir.MatmulPerfMode.DoubleRow,
                        tile_position=(base, 0),
                    )
                return st

            def exp_chunk(st, pt_pair, i):
                """exp(st*scale) -> fp8 plane i%2 of pt_pair, on ACT/DVE/Pool."""
                dst = pair(pt_pair[:])[:, i % 2, :]
                if i in SCHR_DVE or i in SCHR_POOL:
                    it = ipool.tile([128, T2], I32, tag="i32", name="schr")
                    nc.vector.tensor_scalar(
                        it[:],
                        st[:],
                        SCHR_A * scale,
                        SCHR_C,
                        op0=mybir.AluOpType.mult,
                        op1=mybir.AluOpType.add,
                    )
                    eng = nc.vector if i in SCHR_DVE else nc.gpsimd
                    eng.tensor_copy(dst, it[:].bitcast(F32))
                else:
                    nc.scalar.activation(
                        dst, st[:], mybir.ActivationFunctionType.Exp, scale=scale
                    )

            pending = []  # (st_tile, head, sh, i)

            def head(h, sh, filler=None, nxt=None):
                """Attention for one head x query-half, software-pipelined."""
                ft, half = h // 2, (h % 2) * 64
                s0 = sh * T2
                ct = ctpool.tile([128, T2], F32, tag="ct", name="ct")
                sts = pending[:]
                del pending[:]
                while len(sts) < 2:
                    sts.append(st_tile(len(sts), h, sh))
                pt_pair = None
                nissued = 0
                for i in range(TT):
                    st = sts.pop(0)
                    if pt_pair is None:
                        pt_pair = ptpool.tile([128, 2 * T2], F8, tag="pt", name="pt")
                    exp_chunk(st, pt_pair, i)
                    if i + 2 < TT:
                        sts.append(st_tile(i + 2, h, sh))
                    elif nxt is not None and nissued < 2:
                        pending.append(st_tile(nissued, *nxt))
                        nissued += 1
                    if filler is not None:
                        filler(i)
                    if i % 2 == 1:
                        j = i // 2
                        for vt in (v8_sb[j], vr8_sb[j]):
                            lv = vt[:].rearrange(
                                "p (two h x) -> p two h x", two=2, x=VW
                            )[:, :, h, :]
                            for n in range(NSCH2):
                                nc.tensor.matmul(
                                    ct[:, n * SCH2 : (n + 1) * SCH2],
                                    lv,
                                    pair(pt_pair[:])[:, :, n * SCH2 : (n + 1) * SCH2],
                                    start=(j == 0 and vt is v8_sb[j]),
                                    stop=(
                                        j == KP - 1 and vt is vr8_sb[j] and n == NSCH2 - 1
                                    ),
                                    perf_mode=mybir.MatmulPerfMode.DoubleRow,
                                )
                        pt_pair = None

                recip = npool.tile([64, T2], F32, tag="recip", name="recip")
                nc.vector.reciprocal(recip[:], ct[64:128, :])
                nc.vector.tensor_tensor(
                    cn_sb[ft][half : half + 64, s0 : s0 + T2],
                    ct[0:64, :],
                    recip[:],
                    op=mybir.AluOpType.mult,
                )

            def outproj(tiles):
                for i in tiles:
                    osb = osbpool.tile([128, OUT], BF16, tag="osb", name="osb")
                    for oc, ow in OCHUNKS:
                        ps = ppool.tile([128, ow], F32, tag="proj", name="oproj")
                        for f in range(FEAT // 128):
                            nc.tensor.matmul(
                                ps[:],
                                cn_sb[f][:, i * 128 : (i + 1) * 128],
                                wo_sb[f][:, oc : oc + ow],
                                start=(f == 0),
                                stop=(f == FEAT // 128 - 1),
                            )
                        nc.vector.tensor_copy(osb[:, oc : oc + ow], ps[:])
                    nc.sync.dma_start(out_d[i * 128 : (i + 1) * 128, :], osb[:])

            # ---- schedule ----
            # quad groups first (heads 0-3 runnable), then duo while head 0 runs
            for gi in range(4):
                proj_qk(gi)
            for z in range(2):
                pending.append(st_tile(z, 0, 0))
            proj_v(range(2))

            def v_filler(i):
                if i + 2 < TT:
                    proj_v([i + 2])

            # head sequence: (head, sh) pairs; duo projections injected into
            # heads 0-3's windows, outproj dripped into the last heads
            seq = [(2 * p + z, sh) for p in range(3) for sh in range(2) for z in (0, 1)]
            half_tiles = T2 // 128
            pos = 0
            for p in range(3):
                last = p == 2
                for sh in range(2):
                    nxt = seq[pos + 1] if pos + 1 < len(seq) else None
                    head(2 * p, sh, v_filler if (p, sh) == (0, 0) else None, nxt=nxt)
                    pos += 1
                    if p == 0:
                        # duo projection groups qC,kC (sh 0) / qD,kD (sh 1)
                        proj_qk(4 + 2 * sh)
                        proj_qk(5 + 2 * sh)
                    if last and sh == 1:
                        outproj(range(half_tiles // 2, half_tiles))
                    nxt = seq[pos + 1] if pos + 1 < len(seq) else None
                    head(2 * p + 1, sh, nxt=nxt)
                    pos += 1
                    if last and sh == 0:
                        outproj(range(half_tiles // 2))
            outproj(range(half_tiles, TT))

    nc.compile()
    return nc


def _f8(x):
    return np.ascontiguousarray(x).astype(NP_F8)


def _pairs(a):
    """[E, N] -> plane-major pair tiles [E/2, 2N]: rows 256j..256j+128 are
    plane 0, +128..+256 plane 1 of tile j."""
    Erows, N = a.shape
    return np.ascontiguousarray(
        a.reshape(Erows // 256, 2, 128, N).transpose(0, 2, 1, 3).reshape(Erows // 2, 2 * N)
    )


def shard_inputs(query, key, value, wq, bq, wk, bk, wv, bv, wo):
    """Build the 8 per-core input maps (host-side quantize/pack)."""
    # Q/K feature permutation: quad (heads 0-3) planes then duo (heads 4-5)
    perm = np.array(
        [h * DH + d for h in range(4) for d in range(32)]
        + [h * DH + 32 + d for h in range(4) for d in range(32)]
        + [(4 + h) * DH + d for h in range(2) for d in range(32)]
        + [(4 + h) * DH + 32 + d for h in range(2) for d in range(32)]
    )
    in_maps = []
    xs = {}
    for b in range(B):
        m = {}
        for t, arr in (("q", query), ("k", key), ("v", value)):
            xT = np.ascontiguousarray(arr[b].T).astype(np.float32)
            x8 = xT.astype(NP_F8)
            xr8 = (xT - x8.astype(np.float32)).astype(NP_F8)
            m[f"x{t}8"] = _pairs(x8.astype(np.float32)).astype(NP_F8)
            m[f"x{t}r8"] = _pairs(xr8.astype(np.float32)).astype(NP_F8)
        xs[b] = m
    gw = {}
    for g in range(2):
        hs = slice(g * G, (g + 1) * G)
        m = {}
        for t, w_, b_ in (("q", wq, bq), ("k", wk, bk), ("v", wv, bv)):
            wf = (
                w_[hs].transpose(1, 0, 2).reshape(E, G * DH).astype(np.float32)
                * WSCALE
            )
            if t != "v":
                wf = wf[:, perm]
            w8 = wf.astype(NP_F8)
            wr8 = (wf - w8.astype(np.float32)).astype(NP_F8)
            m[f"w{t}8"] = _pairs(w8.astype(np.float32)).astype(NP_F8)
            m[f"w{t}r8"] = _pairs(wr8.astype(np.float32)).astype(NP_F8)
        bqk = np.zeros((128, 8), np.float32)
        bqf = bq[hs].reshape(G * DH)[perm] * WSCALE
        bkf = bk[hs].reshape(G * DH)[perm] * WSCALE
        for gi, (boff, bw) in enumerate(
            [(0, 128), (128, 128), (256, 64), (320, 64)]
        ):
            bqk[:bw, gi] = bqf[boff : boff + bw]
            bqk[:bw, 4 + gi] = bkf[boff : boff + bw]
        # group order in QK_GROUPS: qA,kA,qB,kB,qC,kC,qD,kD -> bias col = gi
        cols = np.zeros((128, 8), np.float32)
        order = [(0, 0), (4, 1), (1, 2), (5, 3), (2, 4), (6, 5), (3, 6), (7, 7)]
        for src, dst in order:
            cols[:, dst] = bqk[:, src]
        m["bqk"] = cols
        m["bv"] = np.ascontiguousarray(
            bv[hs].reshape(1, G * DH) * WSCALE
        ).astype(NP_BF16)
        m["wo"] = np.ascontiguousarray(wo[g * G * DH : (g + 1) * G * DH, :]).astype(
            NP_BF16
        )
        gw[g] = m
    for c in range(N_CORES):
        b, g = c // 2, c % 2
        mm = dict(xs[b])
        mm.update(gw[g])
        in_maps.append(mm)
    return in_maps


_CACHED_NC = None


def kernel(query, key, value, wq, bq, wk, bk, wv, bv, wo, bo):
    global _CACHED_NC
    query, key, value = (np.asarray(a, np.float32) for a in (query, key, value))
    wq, bq, wk, bk, wv, bv, wo, bo = (
        np.asarray(a, np.float32) for a in (wq, bq, wk, bk, wv, bv, wo, bo)
    )
    in_maps = shard_inputs(query, key, value, wq, bq, wk, bk, wv, bv, wo)
    if _CACHED_NC is None:
        _CACHED_NC = build_nc()
    res = run_bass_kernel_spmd(_CACHED_NC, in_maps, list(range(N_CORES)))
    out = np.empty((B, S, E), np.float32)
    for b in range(B):
        out[b] = (
            res.results[2 * b]["out"].astype(np.float32)
            + res.results[2 * b + 1]["out"].astype(np.float32)
            + bo[None, :]
        )
    return out


# revision 12
# speedup vs baseline: 1.1856x; 1.1175x over previous
"""Multi-head attention (B=4, S=2048, E=768, H=12, Dh=64) on 8 TRN2 NeuronCores.

Sharding: batch x head-group tensor parallel. Core c handles batch b = c//2 and
head group g = c%2 (6 heads each). Each core computes its heads' Q/K/V
projections, full attention over the 2048-token sequence, and a partial
out-projection over its 384 concat-features. The host sums the two partials per
batch and adds the output bias.

fp8 DoubleRow design (all matmul inputs fp8e4 except the out-projection):
 - Projections contract E=768 as 3 plane-pairs per term. Weights are
   pre-scaled x64 on host so fp8 sees a normal range; the 1/64 and the bias
   fold into the PSUM->SBUF copy. Weight quantization is compensated with an
   fp8 residual term (x8.w8 + x8.wr8); V additionally compensates x
   quantization (xr8.w8) since V error reaches the output unattenuated.
 - Q/K land directly in fp8 "dh-split" layout: head h occupies 32 partitions,
   planes = dh halves, so the dh=64 score contraction is ONE DoubleRow matmul
   per 128-key x 512-query block (heads 0-3 in a quad tile at partition 32h,
   heads 4-5 in a duo tile).
 - exp runs on ACT emitting fp8 straight into paired P tiles (planes = key
   chunks); chunks {1,5,9,13} of each head-half are offloaded via the
   Schraudolph int32-bitcast exp (DVE scale step, GPSIMD bitcast-copy step)
   to keep ACT off the critical path.
 - PV pairs key chunks per DoubleRow matmul; V carries an additive fp8
   residual pass (vr8 = v - fp8(v)), recovering ~bf16 accuracy at half the
   bf16 PE cost. V is ones-augmented so the matmul emits softmax denominators
   on PSUM partitions 64-127.
 - Out-projection stays bf16; partial outputs ship as bf16 and are upcast and
   summed on the host.

Scheduling: a flat plan of 192 "windows", one exp chunk each (the ACT exp
stream is the critical resource). Score tiles double-buffer 2 windows ahead;
PV pairs, normalizes, V/duo projections and out-projection tiles are placed
into specific windows as filler so the in-order PE stream never blocks on a
not-yet-satisfied dependency.
"""

import math
import os
import sys
from contextlib import ExitStack

import numpy as np

for _p in ("/opt/trn_rl_repo", "/root/.axon_site/_ro/trn_rl_repo"):
    if os.path.isdir(_p) and _p not in sys.path:
        sys.path.append(_p)

os.environ["BASS_NEVER_TRACE"] = "1"

import ml_dtypes  # noqa: E402

import concourse.bass as bass  # noqa: E402
import concourse.tile as tile  # noqa: E402
from concourse import bacc, mybir  # noqa: E402
from concourse.bass_utils import run_bass_kernel_spmd  # noqa: E402

BF16 = mybir.dt.bfloat16
F32 = mybir.dt.float32
F8 = mybir.dt.float8e4
I32 = mybir.dt.int32
NP_BF16 = ml_dtypes.bfloat16
NP_F8 = ml_dtypes.float8_e4m3

B, S, E, H, DH = 4, 2048, 768, 12, 64
N_CORES = 8
G = H // 2  # heads per core (6)
WSCALE = 64.0  # host pre-scale on projection weights (fp8 range)

# per head-half, these key-chunk indices run Schraudolph (DVE + Pool)
SCHR = (1, 5, 9, 13)
SCHR_A = 2**23 / math.log(2)
SCHR_C = float(127 * 2**23 - 0.043677 * 2**23)

DR = mybir.MatmulPerfMode.DoubleRow


def build_nc(T=S, EMB=E, NH=G, dh=DH, OUT=E, trace_label=""):
    """Emit the per-core Bass/Tile program. All cores run this same program."""
    assert T % 128 == 0 and EMB % 256 == 0 and dh == 64 and NH == 6
    FEAT = NH * dh  # 384
    EP = EMB // 256  # E plane-pairs (3)
    TT = T // 128  # key/token tiles (16)
    KP = TT // 2  # key-chunk pairs (8)
    SCH = 512  # projection N chunk
    NSCH = T // SCH
    T2 = T // 2  # attention query-half width (2 PSUM banks)
    SCH2 = 512
    NSCH2 = T2 // SCH2
    FT = FEAT // 128
    OCHUNKS = [(0, OUT // 2), (OUT // 2, OUT // 2)]
    scale = 1.0 / math.sqrt(dh)
    VW = dh + 64  # per-head augmented V width

    # Q/K projection groups: (name, feature col offset, width, plane, qk)
    QK_GROUPS = [
        ("qA", 0, 128, 0, 0), ("kA", 0, 128, 0, 1),
        ("qB", 128, 128, 1, 0), ("kB", 128, 128, 1, 1),
        ("qC", 256, 64, 0, 0), ("kC", 256, 64, 0, 1),
        ("qD", 320, 64, 1, 0), ("kD", 320, 64, 1, 1),
    ]

    nc = bacc.Bacc("TRN2", target_bir_lowering=False, debug=False, num_devices=N_CORES)

    # ---- DRAM I/O ----
    x8_d, w8_d, wr8_d = {}, {}, {}
    for t in ("q", "k", "v"):
        x8_d[t] = nc.dram_tensor(f"x{t}8", [EMB // 2, 2 * T], F8, kind="ExternalInput").ap()
        w8_d[t] = nc.dram_tensor(f"w{t}8", [EMB // 2, 2 * FEAT], F8, kind="ExternalInput").ap()
        wr8_d[t] = nc.dram_tensor(f"w{t}r8", [EMB // 2, 2 * FEAT], F8, kind="ExternalInput").ap()
    xvr8_d = nc.dram_tensor("xvr8", [EMB // 2, 2 * T], F8, kind="ExternalInput").ap()
    wo_d = nc.dram_tensor("wo", [FEAT, OUT], BF16, kind="ExternalInput").ap()
    bqk_d = nc.dram_tensor("bqk", [128, 8], F32, kind="ExternalInput").ap()
    bv_d = nc.dram_tensor("bv", [1, FEAT], BF16, kind="ExternalInput").ap()
    out_d = nc.dram_tensor("out", [T, OUT], BF16, kind="ExternalOutput").ap()

    with tile.TileContext(nc) as tc, ExitStack() as ctx:
        persist = ctx.enter_context(tc.tile_pool(name="persist", bufs=1))

        # ---- persistent SBUF tensors ----
        x8_sb, w8_sb, wr8_sb = {}, {}, {}
        for t in ("q", "k", "v"):
            x8_sb[t] = [persist.tile([128, 2 * T], F8, tag=f"x{t}8_{j}", name=f"x{t}8_{j}") for j in range(EP)]
            w8_sb[t] = [persist.tile([128, 2 * FEAT], F8, tag=f"w{t}8_{j}", name=f"w{t}8_{j}") for j in range(EP)]
            wr8_sb[t] = [persist.tile([128, 2 * FEAT], F8, tag=f"w{t}r8_{j}", name=f"w{t}r8_{j}") for j in range(EP)]
        xvr8_sb = [persist.tile([128, 2 * T], F8, tag=f"xvr8_{j}", name=f"xvr8_{j}") for j in range(EP)]
        wo_sb = [persist.tile([128, OUT], BF16, tag=f"wo{j}", name=f"wo{j}") for j in range(FT)]
        bqk_sb = persist.tile([128, 8], F32, tag="bqk", name="bqk")
        bv_sb = persist.tile([1, FEAT], BF16, tag="bv", name="bv")
        ones_row = persist.tile([1, 128], BF16, tag="ones_row", name="ones_row")
        q8_quad = persist.tile([128, 2 * T], F8, tag="q8_quad", name="q8_quad")
        q8_duo = persist.tile([64, 2 * T], F8, tag="q8_duo", name="q8_duo")
        k8_quad = persist.tile([128, 2 * T], F8, tag="k8_quad", name="k8_quad")
        k8_duo = persist.tile([64, 2 * T], F8, tag="k8_duo", name="k8_duo")
        v8_sb = [persist.tile([128, 2 * NH * VW], F8, tag=f"v8_{j}", name=f"v8_{j}") for j in range(KP)]
        vr8_sb = [persist.tile([128, 2 * NH * VW], F8, tag=f"vr8_{j}", name=f"vr8_{j}") for j in range(KP)]
        cn_sb = [persist.tile([128, T], BF16, tag=f"cn{j}", name=f"cn{j}") for j in range(FT)]

        def pair(ap):  # [p, (2 n)] -> [p, 2, n]
            return ap.rearrange("p (two n) -> p two n", two=2)

        # ---- DMA loads: q path, k path (gates first exp), then v, wo ----
        nc.sync.dma_start(bqk_sb[:], bqk_d[:])
        for t in ("q", "k"):
            for j in range(EP):
                nc.sync.dma_start(w8_sb[t][j][:], w8_d[t][j * 128 : (j + 1) * 128, :])
                nc.sync.dma_start(wr8_sb[t][j][:], wr8_d[t][j * 128 : (j + 1) * 128, :])
                nc.sync.dma_start(x8_sb[t][j][:], x8_d[t][j * 128 : (j + 1) * 128, :])
        nc.sync.dma_start(bv_sb[:], bv_d[:])
        for j in range(EP):
            nc.sync.dma_start(w8_sb["v"][j][:], w8_d["v"][j * 128 : (j + 1) * 128, :])
            nc.sync.dma_start(wr8_sb["v"][j][:], wr8_d["v"][j * 128 : (j + 1) * 128, :])
            nc.sync.dma_start(x8_sb["v"][j][:], x8_d["v"][j * 128 : (j + 1) * 128, :])
            nc.sync.dma_start(xvr8_sb[j][:], xvr8_d[j * 128 : (j + 1) * 128, :])
        for j in range(FT):
            nc.sync.dma_start(wo_sb[j][:], wo_d[j * 128 : (j + 1) * 128, :])
        nc.vector.memset(ones_row[:], 1.0)
        for j in range(KP):
            v8v = v8_sb[j][:].rearrange("p (two h x) -> p two h x", two=2, x=VW)
            nc.gpsimd.memset(v8v[:, :, :, dh:], 1.0)
            vr8v = vr8_sb[j][:].rearrange("p (two h x) -> p two h x", two=2, x=VW)
            nc.gpsimd.memset(vr8v[:, :, :, dh:], 0.0)

        with (
            tc.tile_pool(name="ppsum", bufs=2, space="PSUM") as ppool,
            tc.tile_pool(name="stpsum", bufs=2, space="PSUM") as stpool,
            tc.tile_pool(name="ctpsum", bufs=1, space="PSUM") as ctpool,
            tc.tile_pool(name="ptpool", bufs=10) as ptpool,
            tc.tile_pool(name="i32pool", bufs=2) as ipool,
            tc.tile_pool(name="normpool", bufs=2) as npool,
            tc.tile_pool(name="outsb", bufs=4) as osbpool,
        ):
            qk_dst = {0: (q8_quad, q8_duo), 1: (k8_quad, k8_duo)}

            def proj_qk(gi, n):
                """One N-chunk of one Q/K projection group."""
                name, coff, width, plane, qk = QK_GROUPS[gi]
                t = "q" if qk == 0 else "k"
                ps = ppool.tile([width, SCH], F32, tag="proj", name=name)
                terms = [(x8_sb[t], w8_sb[t]), (x8_sb[t], wr8_sb[t])]
                for ti, (xs, ws) in enumerate(terms):
                    for j in range(EP):
                        nc.tensor.matmul(
                            ps[:],
                            pair(ws[j][:])[:, :, coff : coff + width],
                            pair(xs[j][:])[:, :, n * SCH : (n + 1) * SCH],
                            start=(ti == 0 and j == 0),
                            stop=(ti == len(terms) - 1 and j == EP - 1),
                            perf_mode=DR,
                        )
                quad, duo = qk_dst[qk]
                dst = quad if width == 128 else duo
                dv = pair(dst[:])[:width, plane, n * SCH : (n + 1) * SCH]
                nc.vector.tensor_scalar(
                    dv,
                    ps[:],
                    bqk_sb[:width, gi : gi + 1],
                    1.0 / WSCALE,
                    op0=mybir.AluOpType.add,
                    op1=mybir.AluOpType.mult,
                )

            def proj_v(i):
                """V projection for token tile i; writes v8 + vr8 planes."""
                ps = ppool.tile([128, FEAT], F32, tag="proj", name="vproj")
                nc.tensor.matmul(
                    ps[:], ones_row[:, 0:128], bv_sb[:], start=True, stop=False
                )
                terms = [
                    (x8_sb["v"], w8_sb["v"]),
                    (x8_sb["v"], wr8_sb["v"]),
                    (xvr8_sb, w8_sb["v"]),
                ]
                for ti, (xs, ws) in enumerate(terms):
                    for j in range(EP):
                        nc.tensor.matmul(
                            ps[:],
                            pair(xs[j][:])[:, :, i * 128 : (i + 1) * 128],
                            pair(ws[j][:]),
                            start=False,
                            stop=(ti == 2 and j == EP - 1),
                            perf_mode=DR,
                        )
                pl = i % 2
                psv = ps[:].rearrange("p (h d) -> p h d", d=dh)
                v8v = v8_sb[i // 2][:].rearrange(
                    "p (two h x) -> p two h x", two=2, x=VW
                )[:, pl, :, 0:dh]
                vr8v = vr8_sb[i // 2][:].rearrange(
                    "p (two h x) -> p two h x", two=2, x=VW
                )[:, pl, :, 0:dh]
                nc.vector.tensor_scalar(
                    v8v, psv, 1.0 / WSCALE, None, op0=mybir.AluOpType.mult
                )
                nc.vector.scalar_tensor_tensor(
                    vr8v,
                    psv,
                    1.0 / WSCALE,
                    v8v,
                    op0=mybir.AluOpType.mult,
                    op1=mybir.AluOpType.subtract,
                )

            def st_tile(h, sh, i):
                """Transposed score tile: keys [128i..) x queries half sh."""
                if h < 4:
                    kt, qt, base = k8_quad, q8_quad, 32 * h
                else:
                    kt, qt, base = k8_duo, q8_duo, 32 * (h - 4)
                s0 = sh * T2
                st = stpool.tile([128, T2], F32, tag="st", name="st")
                for n in range(NSCH2):
                    nc.tensor.matmul(
                        st[:, n * SCH2 : (n + 1) * SCH2],
                        pair(kt[:])[base : base + 32, :, i * 128 : (i + 1) * 128],
                        pair(qt[:])[
                            base : base + 32, :, s0 + n * SCH2 : s0 + (n + 1) * SCH2
                        ],
                        start=True,
                        stop=True,
                        perf_mode=DR,
                        tile_position=(base, 0),
                    )
                return st

            def exp_chunk(st, pt_pair, i):
                """exp(st*scale) -> fp8 plane i%2 of pt_pair."""
                dst = pair(pt_pair[:])[:, i % 2, :]
                if i in SCHR:
                    it = ipool.tile([128, T2], I32, tag="i32", name="schr")
                    nc.vector.tensor_scalar(
                        it[:],
                        st[:],
                        SCHR_A * scale,
                        SCHR_C,
                        op0=mybir.AluOpType.mult,
                        op1=mybir.AluOpType.add,
                    )
                    nc.gpsimd.tensor_copy(dst, it[:].bitcast(F32))
                else:
                    nc.scalar.activation(
                        dst, st[:], mybir.ActivationFunctionType.Exp, scale=scale
                    )

            # ---- flat window plan ----
            half_seq = [(h, 0) for h in range(NH)] + [(h, 1) for h in range(NH)]
            NK = len(half_seq)  # 12
            NW = NK * TT  # 192
            fill = [[] for _ in range(NW + 64)]

            # V projections: token tile t in window 11+t (xv DMA lands ~2/3
            # into the q/k stream; these wait on it safely by then)
            for t in range(TT):
                fill[11 + t].append(("vproj", t))
            # duo projection chunks after V (needed from head 4 = window 64)
            duo = [(gi, n) for n in range(NSCH) for gi in (4, 5, 6, 7)]
            for idx, (gi, n) in enumerate(duo):
                fill[28 + idx].append(("qkproj", gi, n))
            # PV pairs: steady density 1 per 2 windows from window 14; the
            # stream trails the exp stream by ~7 pairs, absorbing Pool/DVE
            # exp latency and the V-projection ramp
            pvs = [(k, j) for k in range(NK) for j in range(KP)]
            for idx, (k, j) in enumerate(pvs):
                fill[14 + 2 * idx].append(("pv", k, j))
                if j == KP - 1:
                    fill[15 + 2 * idx].append(("norm", k))
            # out-projection: half 0 tokens after norm(5) (window ~109);
            # half 1 lands in the tail after norm(11)
            for idx in range(TT // 2 * 2):
                fill[112 + 2 * idx].append(("oproj", 0, idx))
            for idx in range(TT // 2 * 2):
                fill[NW + 16 + 2 * idx].append(("oproj", 1, idx))

            ct_for = {}
            pt_for = {}

            def emit_pv(k, j):
                h, sh = half_seq[k]
                if k not in ct_for:
                    ct_for[k] = ctpool.tile([128, T2], F32, tag="ct", name="ct")
                ct = ct_for[k]
                ptp = pt_for.pop((k, j))
                for vt in (v8_sb[j], vr8_sb[j]):
                    lv = vt[:].rearrange("p (two h x) -> p two h x", two=2, x=VW)[
                        :, :, h, :
                    ]
                    for n in range(NSCH2):
                        nc.tensor.matmul(
                            ct[:, n * SCH2 : (n + 1) * SCH2],
                            lv,
                            pair(ptp[:])[:, :, n * SCH2 : (n + 1) * SCH2],
                            start=(j == 0 and vt is v8_sb[j]),
                            stop=(j == KP - 1 and vt is vr8_sb[j] and n == NSCH2 - 1),
                            perf_mode=DR,
                        )

            def emit_norm(k):
                h, sh = half_seq[k]
                ft, half = h // 2, (h % 2) * 64
                s0 = sh * T2
                ct = ct_for.pop(k)
                # split into halves to keep DVE latency jitter small
                for n in range(2):
                    w0 = n * (T2 // 2)
                    recip = npool.tile([64, T2 // 2], F32, tag="recip", name="recip")
                    nc.vector.reciprocal(recip[:], ct[64:128, w0 : w0 + T2 // 2])
                    nc.vector.tensor_tensor(
                        cn_sb[ft][half : half + 64, s0 + w0 : s0 + w0 + T2 // 2],
                        ct[0:64, w0 : w0 + T2 // 2],
                        recip[:],
                        op=mybir.AluOpType.mult,
                    )

            osb_for = {}

            def emit_oproj(sh, idx):
                # idx enumerates (token tile within half, out chunk)
                ti, oc_i = divmod(idx, 2)
                i = sh * (TT // 2) + ti
                oc, ow = OCHUNKS[oc_i]
                if i not in osb_for:
                    osb_for[i] = osbpool.tile([128, OUT], BF16, tag="osb", name="osb")
                osb = osb_for[i]
                ps = ppool.tile([128, ow], F32, tag="proj", name="oproj")
                for f in range(FT):
                    nc.tensor.matmul(
                        ps[:],
                        cn_sb[f][:, i * 128 : (i + 1) * 128],
                        wo_sb[f][:, oc : oc + ow],
                        start=(f == 0),
                        stop=(f == FT - 1),
                    )
                nc.vector.tensor_copy(osb[:, oc : oc + ow], ps[:])
                if oc_i == 1:
                    nc.sync.dma_start(out_d[i * 128 : (i + 1) * 128, :], osb[:])
                    del osb_for[i]

            def emit(task):
                kind = task[0]
                if kind == "vproj":
                    proj_v(task[1])
                elif kind == "qkproj":
                    proj_qk(task[1], task[2])
                elif kind == "pv":
                    emit_pv(task[1], task[2])
                elif kind == "norm":
                    emit_norm(task[1])
                elif kind == "oproj":
                    emit_oproj(task[1], task[2])

            # ---- ramp: quad projection groups (heads 0-3 q/k) ----
            for gi in range(4):
                for n in range(NSCH):
                    proj_qk(gi, n)

            # ---- window loop ----
            sts = [st_tile(*half_seq[0], 0), st_tile(*half_seq[0], 1)]
            for w in range(NW):
                k, i = divmod(w, TT)
                h, sh = half_seq[k]
                st = sts.pop(0)
                if i % 2 == 0:
                    pt_for[(k, i // 2)] = ptpool.tile(
                        [128, 2 * T2], F8, tag="pt", name="pt"
                    )
                exp_chunk(st, pt_for[(k, i // 2)], i)
                nw = w + 2
                if nw < NW:
                    k2, i2 = divmod(nw, TT)
                    sts.append(st_tile(*half_seq[k2], i2))
                for task in fill[w]:
                    emit(task)
            # ---- tail: drain remaining fills ----
            for w in range(NW, len(fill)):
                for task in fill[w]:
                    emit(task)

    nc.compile()
    return nc


def _pairs(a):
    """[E, N] -> plane-major pair tiles [E/2, 2N]: rows 256j..256j+128 are
    plane 0, +128..+256 plane 1 of tile j."""
    Erows, N = a.shape
    return np.ascontiguousarray(
        a.reshape(Erows // 256, 2, 128, N).transpose(0, 2, 1, 3).reshape(Erows // 2, 2 * N)
    )


def shard_inputs(query, key, value, wq, bq, wk, bk, wv, bv, wo):
    """Build the 8 per-core input maps (host-side quantize/pack)."""
    perm = np.array(
        [h * DH + d for h in range(4) for d in range(32)]
        + [h * DH + 32 + d for h in range(4) for d in range(32)]
        + [(4 + h) * DH + d for h in range(2) for d in range(32)]
        + [(4 + h) * DH + 32 + d for h in range(2) for d in range(32)]
    )
    in_maps = []
    xs = {}
    for b in range(B):
        m = {}
        for t, arr in (("q", query), ("k", key), ("v", value)):
            xT = np.ascontiguousarray(arr[b].T).astype(np.float32)
            x8 = xT.astype(NP_F8)
            m[f"x{t}8"] = _pairs(x8.astype(np.float32)).astype(NP_F8)
            if t == "v":
                xr8 = (xT - x8.astype(np.float32)).astype(NP_F8)
                m["xvr8"] = _pairs(xr8.astype(np.float32)).astype(NP_F8)
        xs[b] = m
    gw = {}
    for g in range(2):
        hs = slice(g * G, (g + 1) * G)
        m = {}
        for t, w_ in (("q", wq), ("k", wk), ("v", wv)):
            wf = (
                w_[hs].transpose(1, 0, 2).reshape(E, G * DH).astype(np.float32)
                * WSCALE
            )
            if t != "v":
                wf = wf[:, perm]
            w8 = wf.astype(NP_F8)
            wr8 = (wf - w8.astype(np.float32)).astype(NP_F8)
            m[f"w{t}8"] = _pairs(w8.astype(np.float32)).astype(NP_F8)
            m[f"w{t}r8"] = _pairs(wr8.astype(np.float32)).astype(NP_F8)
        bqf = bq[hs].reshape(G * DH)[perm] * WSCALE
        bkf = bk[hs].reshape(G * DH)[perm] * WSCALE
        cols = np.zeros((128, 8), np.float32)
        # QK_GROUPS order: qA,kA,qB,kB,qC,kC,qD,kD; offsets per group
        for gi, (src, boff, bw) in enumerate(
            [
                (bqf, 0, 128), (bkf, 0, 128),
                (bqf, 128, 128), (bkf, 128, 128),
                (bqf, 256, 64), (bkf, 256, 64),
                (bqf, 320, 64), (bkf, 320, 64),
            ]
        ):
            cols[:bw, gi] = src[boff : boff + bw]
        m["bqk"] = cols
        m["bv"] = np.ascontiguousarray(
            bv[hs].reshape(1, G * DH) * WSCALE
        ).astype(NP_BF16)
        m["wo"] = np.ascontiguousarray(wo[g * G * DH : (g + 1) * G * DH, :]).astype(
            NP_BF16
        )
        gw[g] = m
    for c in range(N_CORES):
        b, g = c // 2, c % 2
        mm = dict(xs[b])
        mm.update(gw[g])
        in_maps.append(mm)
    return in_maps


_CACHED_NC = None


def kernel(query, key, value, wq, bq, wk, bk, wv, bv, wo, bo):
    global _CACHED_NC
    query, key, value = (np.asarray(a, np.float32) for a in (query, key, value))
    wq, bq, wk, bk, wv, bv, wo, bo = (
        np.asarray(a, np.float32) for a in (wq, bq, wk, bk, wv, bv, wo, bo)
    )
    in_maps = shard_inputs(query, key, value, wq, bq, wk, bk, wv, bv, wo)
    if _CACHED_NC is None:
        _CACHED_NC = build_nc()
    res = run_bass_kernel_spmd(_CACHED_NC, in_maps, list(range(N_CORES)))
    out = np.empty((B, S, E), np.float32)
    for b in range(B):
        out[b] = (
            res.results[2 * b]["out"].astype(np.float32)
            + res.results[2 * b + 1]["out"].astype(np.float32)
            + bo[None, :]
        )
    return out
